# revision 25
# baseline (speedup 1.0000x reference)
"""Trainium2 Bass kernel for nn_Attention_14508399525984 (sparse_attention).

Reference computation (B=4, T=1024, C=512, H=8, D=64):
    xn = LN(x, norm_w, norm_b)
    qkv = xn @ qkv_w.T + qkv_b ; q, k, v = split
    q = LN(q, qln_w, qln_b) ; k = LN(k, kln_w, kln_b)
    sim = (q @ k.T) * (D**-0.5) + pair.transpose(0,3,1,2) ; masked += f32min
    out = softmax(sim) @ v ; out @ proj_w.T + proj_b

Sharding: 8 cores = (batch b in 0..3) x (query half ih in 0..1).
Each core gets the full (rolled) batch-b sequence for k/v and its own 512
query rows; outputs are disjoint row blocks of the result.

Device kernel (per core). All matmul operands are fp16 (f32 PSUM accum):
fp16 reduces PE power draw (fp32r runs under a harsher HW power throttle)
and halves the weight/activation DMA. True HW exec time: ~147 us/core
(vs 194 us for the fp32r baseline), NTFF-profiled.
  - host folds norm_w into W2 = diag(norm_w) @ qkv_w.T (biases must be 0)
  - x LN is folded THROUGH the qkv matmul: qkv = rs*(x@W2) - rs*m*colsum(W2),
    realized as raw xT@W2 in PSUM plus a rank-1 (K=1) correction matmul;
    the rs row-scale is dropped for q/k (LN is scale-invariant) and applied
    at v eviction. The q-side matmuls/stats are skipped for the non-query
    half of the sequence (m >= 4). Inputs stream in chunked DMAs so the
    first qkv matmul starts after ~1.5 MB lands instead of ~9 MB.
  - q/k LN via bn_stats/bn_aggr on the PSUM; rsqrt = ACT Sqrt + one batched
    DVE reciprocal per chunk (covers x/k/q at once; a [P,1] DVE reciprocal
    costs ~1 us regardless of width, and the DMA-XBAR transpose path is
    serial ~1.2 us/tile, so both are avoided); normalize on ACT straight to
    fp16; PE fp16 transposes to [c, t]; the qln_w*kln_w*scale product is
    applied per-partition on DVE at q eviction.
  - attention computed transposed: simT[j,i] via K=64 matmuls whose 0/64
    base partitions walrus auto-row-tiles into disjoint PE row groups (both
    heads' sims run concurrently); pair bias + mask tile (comb, fp16,
    host-precomputed with a -4 logit shift so exp fits fp16) added on DVE
    straight to fp16, exp in place on ACT (masked entries are ~-6e4 ->
    exp == 0, no max-subtraction needed); the PV matmul carries a
    ones-column on v giving rows 0..63 = (E@v).T and row 64 = sum_j E.
  - normalize by the sum row: evict to SBUF (ACT), DMA-scatter the row
    across 128 partitions for a cheap DVE reciprocal ([1,512] on a single
    partition costs ~3 us), DMA-gather back, gpsimd partition-broadcast,
    DVE multiply; assemble attn.T [c, i] fp16, project with proj_w.T;
    per-chunk output DMA.
"""

import numpy as np

import concourse.bacc as bacc
import concourse.tile as tile
from concourse import mybir
from concourse.bass_utils import run_bass_kernel_spmd

B, T, C, H, D = 4, 1024, 512, 8, 64
EPS = 1e-5
SCALE = float(D) ** -0.5  # TEMP = 1.0
LOGIT_SHIFT = 4.0  # host subtracts from comb; softmax is shift-invariant
TQ = T // 2  # query rows per core
NCORES = 8
P = 128
F32 = mybir.dt.float32
F16 = mybir.dt.float16

LAST_RESULTS = None  # test harness peeks at this for exec_time_ns


def _build(phases=("ab", "attn", "proj")):
    import os
    phases = tuple(os.environ.get("KPHASES", ",".join(phases)).split(","))
    nc = bacc.Bacc(
        "TRN2",
        target_bir_lowering=False,
        debug=False,
        enable_asserts=False,
        num_devices=NCORES,
    )
    xall_d = nc.declare_dram_parameter("xall", [T, C], F32, isOutput=False)
    xT_d = nc.declare_dram_parameter("xT", [C, T], F16, isOutput=False)
    comb_d = nc.declare_dram_parameter("comb", [H, T, TQ], F16, isOutput=False)
    w2_d = nc.declare_dram_parameter("w2", [C, 3 * C], F16, isOutput=False)
    w2cs_d = nc.declare_dram_parameter("w2cs", [1, 3 * C], F16, isOutput=False)
    wp_d = nc.declare_dram_parameter("wp", [C, C], F16, isOutput=False)
    sc_d = nc.declare_dram_parameter("sc", [C], F32, isOutput=False)
    eye_d = nc.declare_dram_parameter("eye", [P, P], F32, isOutput=False)
    eyeh_d = nc.declare_dram_parameter("eyeh", [P, P], F16, isOutput=False)
    ones_d = nc.declare_dram_parameter("ones", [P, H * H], F16, isOutput=False)
    o_d = nc.declare_dram_parameter("o", [TQ, C], F32, isOutput=True)

    from contextlib import ExitStack

    with tile.TileContext(nc) as tc, ExitStack() as ctx:
        consts = ctx.enter_context(tc.tile_pool(name="consts", bufs=1))
        work = ctx.enter_context(tc.tile_pool(name="work", bufs=4))
        evp = ctx.enter_context(tc.tile_pool(name="evp", bufs=3))

        # tiny consts first, then the big input tiles chunked so the first
        # A/B iteration can start after ~1.5 MB instead of ~9 MB
        ident = consts.tile([P, P], F32)
        nc.sync.dma_start(out=ident, in_=eye_d[:, :])
        identh = consts.tile([P, P], F16)
        nc.sync.dma_start(out=identh, in_=eyeh_d[:, :])
        eps_t = consts.tile([P, 1], F32)
        nc.vector.memset(eps_t, EPS)
        sc_sb = consts.tile([P, 4], F32)
        nc.sync.dma_start(out=sc_sb, in_=sc_d.rearrange("(c p) -> p c", p=P))
        # replicated at partitions 0/32/64 so the three K=1 rank-1
        # matmuls can row-tile (lhsT and rhs must share a base partition)
        w2cs_sb = consts.tile([P // 2 + 1, 3 * C], F16)
        for bp in (0, 32, 64):
            nc.sync.dma_start(out=w2cs_sb[bp : bp + 1, :], in_=w2cs_d[:, :])

        x_sb = consts.tile([P, 8, C], F32)
        xT_sb = consts.tile([P, 4, T], F16)
        w2_sb = consts.tile([P, 4, 3 * C], F16)
        xall_r = xall_d.rearrange("(m p) c -> p m c", p=P)
        xT_r = xT_d.rearrange("(kc p) t -> p kc t", p=P)
        w2_r = w2_d.rearrange("(kc p) n -> p kc n", p=P)
        nc.sync.dma_start(out=x_sb[:, 0, :], in_=xall_r[:, 0, :])
        nc.sync.dma_start(out=xT_sb[:, :, 0:P], in_=xT_r[:, :, 0:P])
        for cc in range(4):
            nc.sync.dma_start(out=w2_sb[:, cc, :], in_=w2_r[:, cc, :])
        for m in range(1, 8):
            ms = slice(m * P, (m + 1) * P)
            nc.sync.dma_start(out=x_sb[:, m, :], in_=xall_r[:, m, :])
            nc.sync.dma_start(out=xT_sb[:, :, ms], in_=xT_r[:, :, ms])

        wp_sb = consts.tile([P, 4, C], F16)
        nc.sync.dma_start(out=wp_sb, in_=wp_d.rearrange("(kc p) n -> p kc n", p=P))

        qT_sb = consts.tile([P, 4, TQ], F16)  # [c, i] query half, sc-scaled
        kT_sb = consts.tile([P, 4, T], F16)  # [c, j]
        v_sb = consts.tile([P, 8, H, D + 1], F16)  # [j_part, jc, h, d | ones]
        nc.sync.dma_start(
            out=v_sb[:, :, :, D],
            in_=ones_d.rearrange("p (a b) -> p a b", a=8),
        )
        attnT_sb = consts.tile([P, 4, TQ], F16)  # [c, i] normalized attn out
        o_sb = consts.tile([P, 4, C], F32)

        # ---------------- phase A/B: LN + qkv + transposes ----------------
        if "ab" in phases:
         with tc.tile_pool(name="pT", bufs=2, space="PSUM") as pT, tc.tile_pool(
            name="pQ", bufs=2, space="PSUM"
        ) as pQ:
            def stats_sd(src, sdall, col, tag):
                """bn stats + sqrt(var+eps) into a shared column of sdall so
                one batched DVE reciprocal serves all stats of the m-chunk
                (a [P,1] DVE reciprocal costs ~1us regardless of width)."""
                st = work.tile([P, 6], F32, name=f"st{tag}")
                nc.vector.bn_stats(st, src)
                mv = work.tile([P, 2], F32, name=f"mv{tag}")
                nc.vector.bn_aggr(mv, st)
                nc.scalar.activation(
                    sdall[:, col : col + 1], mv[:, 1:2],
                    mybir.ActivationFunctionType.Sqrt, bias=eps_t,
                )
                return mv

            def emit_transposes(m, kn, qn):
                ms = slice(m * P, (m + 1) * P)
                ptk = pT.tile([P, 4, P], F16, name="ptk", tag="tp")
                for cc in range(4):
                    nc.tensor.transpose(
                        ptk[:, cc, :], kn[:, cc * P : (cc + 1) * P], identh
                    )
                nc.scalar.copy(out=kT_sb[:, :, ms], in_=ptk)
                if qn is not None:
                    ptq = pT.tile([P, 4, P], F16, name="ptq", tag="tp")
                    for cc in range(4):
                        nc.tensor.transpose(
                            ptq[:, cc, :], qn[:, cc * P : (cc + 1) * P], identh
                        )
                    # eviction + qln_w*kln_w*scale fold in one DVE pass
                    for cc in range(4):
                        nc.vector.tensor_scalar_mul(
                            out=qT_sb[:, cc, ms],
                            in0=ptq[:, cc, :],
                            scalar1=sc_sb[:, cc : cc + 1],
                        )

            pend = None
            for m in range(8):
                ms = slice(m * P, (m + 1) * P)
                has_q = m < 4
                nstat = 3 if has_q else 2
                sdall = work.tile([P, 3], F32, name="sdall")
                rsall = work.tile([P, 3], F32, name="rsall")
                # x row stats for this t-chunk (gates only the rank-1 mms)
                mv = stats_sd(x_sb[:, m, :], sdall, 0, "x")
                negrm = work.tile([P, 1], F32, name="negrm")
                nc.vector.tensor_scalar_mul(out=negrm, in0=mv[:, 0:1], scalar1=-1.0)

                # qkv: psum[t, n] = xT.T @ W2 + negrm x colsum(W2)
                # (missing rs row-scale; q/k LN is scale-invariant, v gets
                # rs at eviction)
                ps_k = pQ.tile([P, C], F32, name="ps_k")
                ps_v = pQ.tile([P, C], F32, name="ps_v")
                ps_q = pQ.tile([P, C], F32, name="ps_q") if has_q else None
                for cc in range(4):
                    lw = xT_sb[:, cc, ms]
                    if has_q:
                        nc.tensor.matmul(
                            ps_q, lw, w2_sb[:, cc, 0:C], start=(cc == 0), stop=False
                        )
                    nc.tensor.matmul(
                        ps_k, lw, w2_sb[:, cc, C : 2 * C], start=(cc == 0), stop=False
                    )
                    nc.tensor.matmul(
                        ps_v, lw, w2_sb[:, cc, 2 * C : 3 * C],
                        start=(cc == 0), stop=False,
                    )
                # negrm as a row [1, 128] for the rank-1 correction;
                # replicate it to partitions 32/64 so the three K=1 matmuls
                # land in distinct PE row groups and run concurrently
                nr_ps = pT.tile([1, P], F32, name="nr_ps", tag="tp")
                nc.tensor.transpose(nr_ps, negrm, ident)
                nr = work.tile([P // 2 + 1, P], F16, name="nr")
                nc.scalar.copy(out=nr[0:1, :], in_=nr_ps)
                nc.sync.dma_start(out=nr[32:33, :], in_=nr[0:1, :])
                nc.sync.dma_start(out=nr[64:65, :], in_=nr[0:1, :])
                if has_q:
                    nc.tensor.matmul(
                        ps_q, nr[0:1, :], w2cs_sb[0:1, 0:C], start=False,
                        stop=True,
                    )
                nc.tensor.matmul(
                    ps_k, nr[32:33, :], w2cs_sb[32:33, C : 2 * C], start=False,
                    stop=True,
                )
                nc.tensor.matmul(
                    ps_v, nr[64:65, :], w2cs_sb[64:65, 2 * C : 3 * C],
                    start=False, stop=True,
                )

                # ---- k path ----
                mvk = stats_sd(ps_k, sdall, 1, "k")
                if has_q:
                    mvq = stats_sd(ps_q, sdall, 2, "q")
                nc.vector.reciprocal(rsall[:, 0:nstat], sdall[:, 0:nstat])
                rs = rsall[:, 0:1]
                rsk = rsall[:, 1:2]
                nmk = work.tile([P, 1], F32, name="nmk")
                nc.vector.tensor_scalar(
                    out=nmk, in0=mvk[:, 0:1], scalar1=rsk, scalar2=-1.0,
                    op0=mybir.AluOpType.mult, op1=mybir.AluOpType.mult,
                )
                kn = evp.tile([P, C], F16, name="kn")
                nc.scalar.activation(
                    kn, ps_k, mybir.ActivationFunctionType.Identity,
                    bias=nmk, scale=rsk,
                )

                qn = None
                if has_q:
                    rsq = rsall[:, 2:3]
                    nmq = work.tile([P, 1], F32, name="nmq")
                    nc.vector.tensor_scalar(
                        out=nmq, in0=mvq[:, 0:1], scalar1=rsq, scalar2=-1.0,
                        op0=mybir.AluOpType.mult, op1=mybir.AluOpType.mult,
                    )
                    qn = evp.tile([P, C], F16, name="qn")
                    nc.scalar.activation(
                        qn, ps_q, mybir.ActivationFunctionType.Identity,
                        bias=nmq, scale=rsq,
                    )

                # ---- v path: v = rs * psum, into [j, jc, h, d] with ones col
                nc.scalar.activation(
                    v_sb[:, m, :, 0:D],
                    ps_v.rearrange("p (h d) -> p h d", h=H),
                    mybir.ActivationFunctionType.Copy,
                    scale=rs,
                )

                # software-pipeline: the transposes wait ~3us on the stats ->
                # normalize chain, and the in-order PE stream would stall the
                # next m's qkv matmuls behind them. Emit the PREVIOUS chunk's
                # transposes here instead (its kn/qn landed during this
                # chunk's matmuls), keeping the PE fed.
                if pend is not None:
                    emit_transposes(*pend)
                pend = (m, kn, qn)

            emit_transposes(*pend)

        # ---------------- attention ----------------
        if "attn" in phases:
         with tc.tile_pool(name="pS", bufs=2, space="PSUM") as pS, tc.tile_pool(
            name="pV", bufs=2, space="PSUM"
        ) as pV, tc.tile_pool(name="combp", bufs=10) as combp, tc.tile_pool(
            name="ep", bufs=3
        ) as ep, tc.tile_pool(name="fin", bufs=2) as fin:
            for hp in range(4):
                h0, h1 = 2 * hp, 2 * hp + 1
                pv0 = pV.tile([D + 1, TQ], F32, name="pv0")
                pv1 = pV.tile([D + 1, TQ], F32, name="pv1")
                for jc in range(8):
                    js = slice(jc * P, (jc + 1) * P)
                    # both heads of the pair batched into one wide tile:
                    # one DMA, one DVE add, one ACT exp per (hp, jc)
                    cmb = combp.tile([P, 2, TQ], F16, name="cmb")
                    nc.sync.dma_start(
                        out=cmb,
                        in_=comb_d[h0 : h0 + 2, js, :].transpose([1, 0, 2]),
                    )
                    sim = pS.tile([P, 2, TQ], F32, name="sim")
                    # K=64 each with base partitions 0/64: walrus row-tiles
                    # the pair into disjoint PE row-groups automatically, so
                    # both heads' sims run concurrently
                    for idx in range(2):
                        lo, hi = (0, D) if idx == 0 else (D, 2 * D)
                        nc.tensor.matmul(
                            sim[:, idx, :],
                            kT_sb[lo:hi, hp, js],
                            qT_sb[lo:hi, hp, :],
                            start=True,
                            stop=True,
                        )
                    # add straight to fp16, exp in place (fp16 logits are
                    # fine: |logit| <= ~10, abs err ~5e-3)
                    et = ep.tile([P, 2, TQ], F16, name="et")
                    nc.vector.tensor_add(out=et, in0=sim, in1=cmb)
                    nc.scalar.activation(
                        et, et, mybir.ActivationFunctionType.Exp
                    )
                    for idx, (h, pvt) in enumerate(((h0, pv0), (h1, pv1))):
                        nc.tensor.matmul(
                            pvt,
                            v_sb[:, jc, h, :],
                            et[:, idx, :],
                            start=(jc == 0),
                            stop=(jc == 7),
                        )
                # finalize both heads: divide by the sum row
                for idx, pvt in enumerate((pv0, pv1)):
                    # evict the sum row to SBUF (ACT, stays on partition
                    # 64), DMA-scatter it across 128 partitions for the
                    # reciprocal (a [1,512] single-partition DVE op costs
                    # ~3us; [128,4] costs ~0.1us), DMA-gather back to a
                    # row, broadcast on gpsimd
                    srow = fin.tile([D + 1, TQ], F32, name=f"srow{idx}")
                    nc.scalar.copy(out=srow[D : D + 1, :], in_=pvt[D : D + 1, :])
                    s4 = fin.tile([P, 4], F32, name=f"s4{idx}")
                    nc.sync.dma_start(out=s4, in_=srow[D : D + 1, :])
                    r4 = fin.tile([P, 4], F32, name=f"r4{idx}")
                    nc.vector.reciprocal(r4, s4)
                    r0 = fin.tile([1, TQ], F32, name=f"r0{idx}")
                    nc.sync.dma_start(out=r0, in_=r4)
                    rb = fin.tile([D, TQ], F32, name=f"rb{idx}")
                    nc.gpsimd.partition_broadcast(rb, r0)
                    if idx == 0:
                        nc.vector.tensor_mul(
                            out=attnT_sb[0:D, hp, :], in0=pvt[0:D, :], in1=rb
                        )
                    else:
                        tmo = fin.tile([D, TQ], F16, name="tmo")
                        nc.vector.tensor_mul(out=tmo, in0=pvt[0:D, :], in1=rb)
                        nc.sync.dma_start(out=attnT_sb[D:P, hp, :], in_=tmo)

        # ---------------- projection ----------------
        if "proj" in phases:
         with tc.tile_pool(name="pO", bufs=1, space="PSUM") as pO:
            o_r = o_d.rearrange("(ic p) n -> p ic n", p=P)
            # cc (= head pair) as the OUTER loop over 4 live PSUM tiles:
            # 12 of the 16 matmuls depend only on head pairs 0..2 and run
            # while the last head pair's softmax finalize is still in
            # flight (ic-outer order would stall the in-order PE stream on
            # the first ic's cc=3 matmul)
            po_t = [pO.tile([P, C], F32, name=f"po{ic}") for ic in range(4)]
            for cc in range(4):
                for ic in range(4):
                    nc.tensor.matmul(
                        po_t[ic],
                        attnT_sb[:, cc, ic * P : (ic + 1) * P],
                        wp_sb[:, cc, :],
                        start=(cc == 0),
                        stop=(cc == 3),
                    )
                    if cc == 3:
                        nc.scalar.copy(out=o_sb[:, ic, :], in_=po_t[ic])
                        nc.sync.dma_start(out=o_r[:, ic, :], in_=o_sb[:, ic, :])

    nc.compile()
    return nc


def _make_runner(nc, donate=True, scan_n=0):
    """Mirror of bass2jax.run_bass_via_pjrt that returns a reusable jitted
    callable (so the harness can time repeated executions on-device).

    scan_n > 0 chains scan_n sequential executions of the NEFF inside one
    dispatch (the output feeds the next iteration's output-donation operand),
    letting wall-clock deltas isolate the per-execution device time from the
    axon dispatch overhead."""
    import jax
    import numpy as _np
    from jax.experimental.shard_map import shard_map
    from jax.sharding import Mesh, PartitionSpec

    from concourse.bass2jax import (
        _bass_exec_p,
        install_neuronx_cc_hook,
        partition_id_tensor,
    )

    install_neuronx_cc_hook()
    partition_name = nc.partition_id_tensor.name if nc.partition_id_tensor else None

    in_names, out_names, out_avals, zero_outs = [], [], [], []
    for alloc in nc.m.functions[0].allocations:
        if not isinstance(alloc, mybir.MemoryLocationSet):
            continue
        name = alloc.memorylocations[0].name
        if alloc.kind == "ExternalInput":
            if name != partition_name:
                in_names.append(name)
        elif alloc.kind == "ExternalOutput":
            shape = tuple(alloc.tensor_shape)
            dtype = mybir.dt.np(alloc.dtype)
            out_names.append(name)
            out_avals.append(jax.core.ShapedArray(shape, dtype))
            zero_outs.append(_np.zeros(shape, dtype))
    n_params = len(in_names)
    n_outs = len(out_avals)
    all_in_names = list(in_names) + list(out_names)
    if partition_name is not None:
        all_in_names.append(partition_name)

    def _call(operands):
        if partition_name is not None:
            operands = operands + [partition_id_tensor()]
        return _bass_exec_p.bind(
            *operands,
            out_avals=tuple(out_avals),
            in_names=tuple(all_in_names),
            out_names=tuple(out_names),
            lowering_input_output_aliases=(),
            sim_require_finite=True,
            sim_require_nnan=True,
            nc=nc,
        )

    if scan_n:
        assert n_outs == 1, "scan timing mode assumes a single output"

        def _body(*args):
            ins, carry = list(args[:n_params]), args[n_params]
            for _ in range(scan_n):
                (carry,) = _call(ins + [carry])
            return (carry,)

    else:

        def _body(*args):
            return tuple(_call(list(args)))

    devices = jax.devices()[:NCORES]
    mesh = Mesh(_np.asarray(devices), ("core",))
    in_specs = (PartitionSpec("core"),) * (n_params + n_outs)
    out_specs = (PartitionSpec("core"),) * n_outs
    jit_kwargs = dict(keep_unused=True)
    if donate:
        jit_kwargs["donate_argnums"] = tuple(range(n_params, n_params + n_outs))
    fn = jax.jit(
        shard_map(_body, mesh=mesh, in_specs=in_specs, out_specs=out_specs,
                  check_rep=False),
        **jit_kwargs,
    )

    def prep(in_maps):
        concat_in = [
            _np.concatenate([_np.asarray(m[name]) for m in in_maps], axis=0)
            for name in in_names
        ]
        concat_zeros = [
            _np.zeros((NCORES * z.shape[0], *z.shape[1:]), z.dtype)
            for z in zero_outs
        ]
        return concat_in, concat_zeros

    def unpack(out_arrs):
        return [
            {
                name: _np.asarray(out_arrs[i]).reshape(
                    NCORES, *out_avals[i].shape
                )[c]
                for i, name in enumerate(out_names)
            }
            for c in range(NCORES)
        ]

    return fn, prep, unpack


def kernel(
    x, pair, mask, norm_w, norm_b, qkv_w, qkv_b, qln_w, qln_b, kln_w, kln_b,
    proj_w, proj_b,
):
    global LAST_RESULTS
    x = np.asarray(x, dtype=np.float32)
    pair = np.asarray(pair, dtype=np.float32)
    mask = np.asarray(mask)
    f32 = np.float32
    f16 = np.float16

    # host-side weight folding
    w2 = (np.asarray(qkv_w, f32).T * np.asarray(norm_w, f32)[:, None]).astype(f32)
    b2 = np.asarray(qkv_b, f32) + np.asarray(norm_b, f32) @ np.asarray(qkv_w, f32).T
    assert np.all(b2 == 0.0), "nonzero effective qkv bias not supported"
    assert np.all(np.asarray(qln_b) == 0.0) and np.all(np.asarray(kln_b) == 0.0), (
        "nonzero q/k LN bias not supported"
    )
    assert np.all(np.asarray(proj_b) == 0.0), "nonzero proj bias not supported"
    w2cs = np.ascontiguousarray(
        w2.sum(axis=0, dtype=np.float64).astype(f32)[None, :]
    ).astype(f16)
    wp = np.ascontiguousarray(np.asarray(proj_w, f32).T).astype(f16)
    sc = (np.asarray(qln_w, f32) * np.asarray(kln_w, f32) * f32(SCALE)).astype(f32)
    w2h = w2.astype(f16)

    neg = np.float32(np.finfo(np.float32).min)
    in_maps = []
    for core in range(NCORES):
        b, ih = divmod(core, 2)
        i0 = ih * TQ
        # roll the sequence so this core's query rows are rows 0..TQ-1
        xb = np.concatenate([x[b, i0:], x[b, :i0]], axis=0)
        xT = np.ascontiguousarray(xb.T).astype(f16)
        # comb[h, j, i] = pair[b, i0+i, j, h] + (mask ? 0 : f32min), j rolled
        comb = np.ascontiguousarray(pair[b, i0 : i0 + TQ].transpose(2, 1, 0))
        mb = np.where(mask[b, i0 : i0 + TQ], f32(0.0), neg).T  # [j, i]
        comb += mb[None, :, :]
        comb = np.concatenate([comb[:, i0:, :], comb[:, :i0, :]], axis=1)
        # global logit shift (softmax-invariant) so exp() stays in fp16
        # range; masked entries clamp to a sentinel that still underflows
        # exp() to exactly 0 after +qk
        comb -= f32(LOGIT_SHIFT)
        comb = np.maximum(comb, -60000.0).astype(f16)
        comb = np.ascontiguousarray(comb)
        in_maps.append(
            {
                "xall": xb,
                "xT": xT,
                "comb": comb,
                "w2": w2h,
                "w2cs": w2cs,
                "wp": wp,
                "sc": sc,
                "eye": np.eye(P, dtype=f32),
                "eyeh": np.eye(P, dtype=f16),
                "ones": np.ones((P, H * H), f16),
            }
        )

    nc = _build()
    fn, prep, unpack = _make_runner(nc, donate=False)
    concat_in, concat_zeros = prep(in_maps)
    results = unpack(fn(*concat_in, *concat_zeros))
    LAST_RESULTS = {
        "nc": nc,
        "in_maps": in_maps,
        "fn": fn,
        "concat_in": concat_in,
        "concat_zeros": concat_zeros,
    }

    out = np.empty((B, T, C), dtype=np.float32)
    for core in range(NCORES):
        b, ih = divmod(core, 2)
        out[b, ih * TQ : (ih + 1) * TQ] = results[core]["o"]
    return out


# revision 26
# speedup vs baseline: 1.1746x; 1.1746x over previous
"""Trainium2 Bass kernel for nn_Attention_14508399525984 (sparse_attention).

Reference computation (B=4, T=1024, C=512, H=8, D=64):
    xn = LN(x, norm_w, norm_b)
    qkv = xn @ qkv_w.T + qkv_b ; q, k, v = split
    q = LN(q, qln_w, qln_b) ; k = LN(k, kln_w, kln_b)
    sim = (q @ k.T) * (D**-0.5) + pair.transpose(0,3,1,2) ; masked += f32min
    out = softmax(sim) @ v ; out @ proj_w.T + proj_b

Sharding: 8 cores = (batch b in 0..3) x (query half ih in 0..1).
Each core gets the full (rolled) batch-b sequence for k/v and its own 512
query rows; outputs are disjoint row blocks of the result.

Device kernel (per core). All matmul operands are fp16 (f32 PSUM accum):
fp16 reduces PE power draw (fp32r runs under a harsher HW power throttle)
and halves the weight/activation DMA. True HW exec time: ~147 us/core
(vs 194 us for the fp32r baseline), NTFF-profiled.
  - host folds norm_w into W2 = diag(norm_w) @ qkv_w.T (biases must be 0)
  - x LN is folded THROUGH the qkv matmul: qkv = rs*(x@W2) - rs*m*colsum(W2),
    realized as raw xT@W2 in PSUM plus a rank-1 (K=1) correction matmul;
    the rs row-scale is dropped for q/k (LN is scale-invariant) and applied
    at v eviction. The q-side matmuls/stats are skipped for the non-query
    half of the sequence (m >= 4). Inputs stream in chunked DMAs so the
    first qkv matmul starts after ~1.5 MB lands instead of ~9 MB.
  - q/k LN via bn_stats/bn_aggr on the PSUM; rsqrt = ACT Sqrt + one batched
    DVE reciprocal per chunk (covers x/k/q at once; a [P,1] DVE reciprocal
    costs ~1 us regardless of width, and the DMA-XBAR transpose path is
    serial ~1.2 us/tile, so both are avoided); normalize on ACT straight to
    fp16; PE fp16 transposes to [c, t]; the qln_w*kln_w*scale product is
    applied per-partition on DVE at q eviction.
  - attention computed transposed: simT[j,i] via K=64 matmuls whose 0/64
    base partitions walrus auto-row-tiles into disjoint PE row groups (both
    heads' sims run concurrently); pair bias + mask tile (comb, fp16,
    host-precomputed with a -4 logit shift so exp fits fp16) added on DVE
    straight to fp16, exp in place on ACT (masked entries are ~-6e4 ->
    exp == 0, no max-subtraction needed); the PV matmul carries a
    ones-column on v giving rows 0..63 = (E@v).T and row 64 = sum_j E.
  - normalize by the sum row: evict to SBUF (ACT), DMA-scatter the row
    across 128 partitions for a cheap DVE reciprocal ([1,512] on a single
    partition costs ~3 us), DMA-gather back, gpsimd partition-broadcast,
    DVE multiply; assemble attn.T [c, i] fp16, project with proj_w.T;
    per-chunk output DMA.
"""

import numpy as np

import concourse.bacc as bacc
import concourse.tile as tile
from concourse import mybir
from concourse.bass_utils import run_bass_kernel_spmd

B, T, C, H, D = 4, 1024, 512, 8, 64
EPS = 1e-5
SCALE = float(D) ** -0.5  # TEMP = 1.0
LOGIT_SHIFT = 4.0  # host subtracts from comb; softmax is shift-invariant
TQ = T // 2  # query rows per core
NCORES = 8
P = 128
F32 = mybir.dt.float32
F16 = mybir.dt.float16

LAST_RESULTS = None  # test harness peeks at this for exec_time_ns


def _build(phases=("ab", "attn", "proj")):
    import os
    phases = tuple(os.environ.get("KPHASES", ",".join(phases)).split(","))
    nc = bacc.Bacc(
        "TRN2",
        target_bir_lowering=False,
        debug=False,
        enable_asserts=False,
        num_devices=NCORES,
    )
    xall_d = nc.declare_dram_parameter("xall", [T, C], F32, isOutput=False)
    xT_d = nc.declare_dram_parameter("xT", [C, T], F16, isOutput=False)
    comb_d = nc.declare_dram_parameter("comb", [H, T, TQ], F16, isOutput=False)
    w2_d = nc.declare_dram_parameter("w2", [C, 3 * C], F16, isOutput=False)
    w2cs_d = nc.declare_dram_parameter("w2cs", [1, 3 * C], F16, isOutput=False)
    wp_d = nc.declare_dram_parameter("wp", [C, C], F16, isOutput=False)
    sc_d = nc.declare_dram_parameter("sc", [C], F32, isOutput=False)
    eye_d = nc.declare_dram_parameter("eye", [P, P], F32, isOutput=False)
    eyeh_d = nc.declare_dram_parameter("eyeh", [P, P], F16, isOutput=False)
    ones_d = nc.declare_dram_parameter("ones", [P, H * H], F16, isOutput=False)
    o_d = nc.declare_dram_parameter("o", [TQ, C], F32, isOutput=True)

    from contextlib import ExitStack

    with tile.TileContext(nc) as tc, ExitStack() as ctx:
        consts = ctx.enter_context(tc.tile_pool(name="consts", bufs=1))
        work = ctx.enter_context(tc.tile_pool(name="work", bufs=4))
        evp = ctx.enter_context(tc.tile_pool(name="evp", bufs=3))

        # tiny consts first, then the big input tiles chunked so the first
        # A/B iteration can start after ~1.5 MB instead of ~9 MB
        ident = consts.tile([P, P], F32)
        nc.sync.dma_start(out=ident, in_=eye_d[:, :])
        identh = consts.tile([P, P], F16)
        nc.sync.dma_start(out=identh, in_=eyeh_d[:, :])
        eps_t = consts.tile([P, 1], F32)
        nc.vector.memset(eps_t, EPS)
        sc_sb = consts.tile([P, 4], F32)
        nc.sync.dma_start(out=sc_sb, in_=sc_d.rearrange("(c p) -> p c", p=P))
        w2cs_sb = consts.tile([1, 3 * C], F16)
        nc.sync.dma_start(out=w2cs_sb, in_=w2cs_d[:, :])

        x_sb = consts.tile([P, 8, C], F32)
        xT_sb = consts.tile([P, 4, T], F16)
        w2_sb = consts.tile([P, 4, 3 * C], F16)
        xall_r = xall_d.rearrange("(m p) c -> p m c", p=P)
        xT_r = xT_d.rearrange("(kc p) t -> p kc t", p=P)
        w2_r = w2_d.rearrange("(kc p) n -> p kc n", p=P)
        nc.sync.dma_start(out=x_sb[:, 0, :], in_=xall_r[:, 0, :])
        nc.sync.dma_start(out=xT_sb[:, :, 0:P], in_=xT_r[:, :, 0:P])
        for cc in range(4):
            nc.sync.dma_start(out=w2_sb[:, cc, :], in_=w2_r[:, cc, :])
        for m in range(1, 8):
            ms = slice(m * P, (m + 1) * P)
            nc.sync.dma_start(out=x_sb[:, m, :], in_=xall_r[:, m, :])
            nc.sync.dma_start(out=xT_sb[:, :, ms], in_=xT_r[:, :, ms])

        wp_sb = consts.tile([P, 4, C], F16)
        nc.sync.dma_start(out=wp_sb, in_=wp_d.rearrange("(kc p) n -> p kc n", p=P))

        qT_sb = consts.tile([P, 4, TQ], F16)  # [c, i] query half, sc-scaled
        kT_sb = consts.tile([P, 4, T], F16)  # [c, j]
        v_sb = consts.tile([P, 8, H, D + 1], F16)  # [j_part, jc, h, d | ones]
        nc.sync.dma_start(
            out=v_sb[:, :, :, D],
            in_=ones_d.rearrange("p (a b) -> p a b", a=8),
        )
        attnT_sb = consts.tile([P, 4, TQ], F16)  # [c, i] normalized attn out
        o_sb = consts.tile([P, 4, C], F32)

        # ---------------- phase A/B: LN + qkv + transposes ----------------
        if "ab" in phases:
         with tc.tile_pool(name="pT", bufs=2, space="PSUM") as pT, tc.tile_pool(
            name="pQ", bufs=2, space="PSUM"
        ) as pQ:
            def stats_sd(src, sdall, col, tag):
                """bn stats + sqrt(var+eps) into a shared column of sdall so
                one batched DVE reciprocal serves all stats of the m-chunk
                (a [P,1] DVE reciprocal costs ~1us regardless of width)."""
                st = work.tile([P, 6], F32, name=f"st{tag}")
                nc.vector.bn_stats(st, src)
                mv = work.tile([P, 2], F32, name=f"mv{tag}")
                nc.vector.bn_aggr(mv, st)
                nc.scalar.activation(
                    sdall[:, col : col + 1], mv[:, 1:2],
                    mybir.ActivationFunctionType.Sqrt, bias=eps_t,
                )
                return mv

            def emit_transposes(m, kn, qn):
                ms = slice(m * P, (m + 1) * P)
                ptk = pT.tile([P, 4, P], F16, name="ptk", tag="tp")
                for cc in range(4):
                    nc.tensor.transpose(
                        ptk[:, cc, :], kn[:, cc * P : (cc + 1) * P], identh
                    )
                nc.scalar.copy(out=kT_sb[:, :, ms], in_=ptk)
                if qn is not None:
                    ptq = pT.tile([P, 4, P], F16, name="ptq", tag="tp")
                    for cc in range(4):
                        nc.tensor.transpose(
                            ptq[:, cc, :], qn[:, cc * P : (cc + 1) * P], identh
                        )
                    # eviction + qln_w*kln_w*scale fold in one DVE pass
                    for cc in range(4):
                        nc.vector.tensor_scalar_mul(
                            out=qT_sb[:, cc, ms],
                            in0=ptq[:, cc, :],
                            scalar1=sc_sb[:, cc : cc + 1],
                        )

            pend = None
            for m in range(8):
                ms = slice(m * P, (m + 1) * P)
                has_q = m < 4
                nstat = 3 if has_q else 2
                sdall = work.tile([P, 3], F32, name="sdall")
                rsall = work.tile([P, 3], F32, name="rsall")
                # x row stats for this t-chunk (gates only the rank-1 mms)
                mv = stats_sd(x_sb[:, m, :], sdall, 0, "x")
                negrm = work.tile([P, 1], F32, name="negrm")
                nc.vector.tensor_scalar_mul(out=negrm, in0=mv[:, 0:1], scalar1=-1.0)

                # qkv: psum[t, n] = xT.T @ W2 + negrm x colsum(W2)
                # (missing rs row-scale; q/k LN is scale-invariant, v gets
                # rs at eviction)
                ps_k = pQ.tile([P, C], F32, name="ps_k")
                ps_v = pQ.tile([P, C], F32, name="ps_v")
                ps_q = pQ.tile([P, C], F32, name="ps_q") if has_q else None
                for cc in range(4):
                    lw = xT_sb[:, cc, ms]
                    if has_q:
                        nc.tensor.matmul(
                            ps_q, lw, w2_sb[:, cc, 0:C], start=(cc == 0), stop=False
                        )
                    nc.tensor.matmul(
                        ps_k, lw, w2_sb[:, cc, C : 2 * C], start=(cc == 0), stop=False
                    )
                    nc.tensor.matmul(
                        ps_v, lw, w2_sb[:, cc, 2 * C : 3 * C],
                        start=(cc == 0), stop=False,
                    )
                # negrm as a row [1, 128] for the rank-1 correction
                nr_ps = pT.tile([1, P], F32, name="nr_ps", tag="tp")
                nc.tensor.transpose(nr_ps, negrm, ident)
                nr = work.tile([1, P], F16, name="nr")
                nc.scalar.copy(out=nr, in_=nr_ps)
                if has_q:
                    nc.tensor.matmul(
                        ps_q, nr, w2cs_sb[:, 0:C], start=False, stop=True
                    )
                nc.tensor.matmul(
                    ps_k, nr, w2cs_sb[:, C : 2 * C], start=False, stop=True
                )
                nc.tensor.matmul(
                    ps_v, nr, w2cs_sb[:, 2 * C : 3 * C], start=False, stop=True
                )

                # ---- k path ----
                mvk = stats_sd(ps_k, sdall, 1, "k")
                if has_q:
                    mvq = stats_sd(ps_q, sdall, 2, "q")
                nc.vector.reciprocal(rsall[:, 0:nstat], sdall[:, 0:nstat])
                rs = rsall[:, 0:1]
                rsk = rsall[:, 1:2]
                nmk = work.tile([P, 1], F32, name="nmk")
                nc.vector.tensor_scalar(
                    out=nmk, in0=mvk[:, 0:1], scalar1=rsk, scalar2=-1.0,
                    op0=mybir.AluOpType.mult, op1=mybir.AluOpType.mult,
                )
                kn = evp.tile([P, C], F16, name="kn")
                nc.scalar.activation(
                    kn, ps_k, mybir.ActivationFunctionType.Identity,
                    bias=nmk, scale=rsk,
                )

                qn = None
                if has_q:
                    rsq = rsall[:, 2:3]
                    nmq = work.tile([P, 1], F32, name="nmq")
                    nc.vector.tensor_scalar(
                        out=nmq, in0=mvq[:, 0:1], scalar1=rsq, scalar2=-1.0,
                        op0=mybir.AluOpType.mult, op1=mybir.AluOpType.mult,
                    )
                    qn = evp.tile([P, C], F16, name="qn")
                    nc.scalar.activation(
                        qn, ps_q, mybir.ActivationFunctionType.Identity,
                        bias=nmq, scale=rsq,
                    )

                # ---- v path: v = rs * psum, into [j, jc, h, d] with ones col
                nc.scalar.activation(
                    v_sb[:, m, :, 0:D],
                    ps_v.rearrange("p (h d) -> p h d", h=H),
                    mybir.ActivationFunctionType.Copy,
                    scale=rs,
                )

                # software-pipeline: the transposes wait ~3us on the stats ->
                # normalize chain, and the in-order PE stream would stall the
                # next m's qkv matmuls behind them. Emit the PREVIOUS chunk's
                # transposes here instead (its kn/qn landed during this
                # chunk's matmuls), keeping the PE fed.
                if pend is not None:
                    emit_transposes(*pend)
                pend = (m, kn, qn)

            emit_transposes(*pend)

        # ---------------- attention ----------------
        if "attn" in phases:
         with tc.tile_pool(name="pS", bufs=2, space="PSUM") as pS, tc.tile_pool(
            name="pV", bufs=2, space="PSUM"
        ) as pV, tc.tile_pool(name="combp", bufs=10) as combp, tc.tile_pool(
            name="ep", bufs=3
        ) as ep, tc.tile_pool(name="fin", bufs=2) as fin:
            for hp in range(4):
                h0, h1 = 2 * hp, 2 * hp + 1
                pv0 = pV.tile([D + 1, TQ], F32, name="pv0")
                pv1 = pV.tile([D + 1, TQ], F32, name="pv1")
                for jc in range(8):
                    js = slice(jc * P, (jc + 1) * P)
                    # both heads of the pair batched into one wide tile:
                    # one DMA, one DVE add, one ACT exp per (hp, jc)
                    cmb = combp.tile([P, 2, TQ], F16, name="cmb")
                    nc.sync.dma_start(
                        out=cmb,
                        in_=comb_d[h0 : h0 + 2, js, :].transpose([1, 0, 2]),
                    )
                    sim = pS.tile([P, 2, TQ], F32, name="sim")
                    # K=64 each with base partitions 0/64: walrus row-tiles
                    # the pair into disjoint PE row-groups automatically, so
                    # both heads' sims run concurrently
                    for idx in range(2):
                        lo, hi = (0, D) if idx == 0 else (D, 2 * D)
                        nc.tensor.matmul(
                            sim[:, idx, :],
                            kT_sb[lo:hi, hp, js],
                            qT_sb[lo:hi, hp, :],
                            start=True,
                            stop=True,
                        )
                    # add straight to fp16, exp in place (fp16 logits are
                    # fine: |logit| <= ~10, abs err ~5e-3)
                    et = ep.tile([P, 2, TQ], F16, name="et")
                    nc.vector.tensor_add(out=et, in0=sim, in1=cmb)
                    nc.scalar.activation(
                        et, et, mybir.ActivationFunctionType.Exp
                    )
                    for idx, (h, pvt) in enumerate(((h0, pv0), (h1, pv1))):
                        nc.tensor.matmul(
                            pvt,
                            v_sb[:, jc, h, :],
                            et[:, idx, :],
                            start=(jc == 0),
                            stop=(jc == 7),
                        )
                # finalize both heads: divide by the sum row
                for idx, pvt in enumerate((pv0, pv1)):
                    # evict the sum row to SBUF (ACT, stays on partition
                    # 64), DMA-scatter it across 128 partitions for the
                    # reciprocal (a [1,512] single-partition DVE op costs
                    # ~3us; [128,4] costs ~0.1us), DMA-gather back to a
                    # row, broadcast on gpsimd
                    srow = fin.tile([D + 1, TQ], F32, name=f"srow{idx}")
                    nc.scalar.copy(out=srow[D : D + 1, :], in_=pvt[D : D + 1, :])
                    s4 = fin.tile([P, 4], F32, name=f"s4{idx}")
                    nc.sync.dma_start(out=s4, in_=srow[D : D + 1, :])
                    r4 = fin.tile([P, 4], F32, name=f"r4{idx}")
                    nc.vector.reciprocal(r4, s4)
                    r0 = fin.tile([1, TQ], F32, name=f"r0{idx}")
                    nc.sync.dma_start(out=r0, in_=r4)
                    rb = fin.tile([D, TQ], F32, name=f"rb{idx}")
                    nc.gpsimd.partition_broadcast(rb, r0)
                    if idx == 0:
                        nc.vector.tensor_mul(
                            out=attnT_sb[0:D, hp, :], in0=pvt[0:D, :], in1=rb
                        )
                    else:
                        tmo = fin.tile([D, TQ], F16, name="tmo")
                        nc.vector.tensor_mul(out=tmo, in0=pvt[0:D, :], in1=rb)
                        nc.sync.dma_start(out=attnT_sb[D:P, hp, :], in_=tmo)

        # ---------------- projection ----------------
        if "proj" in phases:
         with tc.tile_pool(name="pO", bufs=1, space="PSUM") as pO:
            o_r = o_d.rearrange("(ic p) n -> p ic n", p=P)
            # cc (= head pair) as the OUTER loop over 4 live PSUM tiles:
            # 12 of the 16 matmuls depend only on head pairs 0..2 and run
            # while the last head pair's softmax finalize is still in
            # flight (ic-outer order would stall the in-order PE stream on
            # the first ic's cc=3 matmul)
            po_t = [pO.tile([P, C], F32, name=f"po{ic}") for ic in range(4)]
            for cc in range(4):
                for ic in range(4):
                    nc.tensor.matmul(
                        po_t[ic],
                        attnT_sb[:, cc, ic * P : (ic + 1) * P],
                        wp_sb[:, cc, :],
                        start=(cc == 0),
                        stop=(cc == 3),
                    )
                    if cc == 3:
                        nc.scalar.copy(out=o_sb[:, ic, :], in_=po_t[ic])
                        nc.sync.dma_start(out=o_r[:, ic, :], in_=o_sb[:, ic, :])

    nc.compile()
    return nc


def _make_runner(nc, donate=True, scan_n=0):
    """Mirror of bass2jax.run_bass_via_pjrt that returns a reusable jitted
    callable (so the harness can time repeated executions on-device).

    scan_n > 0 chains scan_n sequential executions of the NEFF inside one
    dispatch (the output feeds the next iteration's output-donation operand),
    letting wall-clock deltas isolate the per-execution device time from the
    axon dispatch overhead."""
    import jax
    import numpy as _np
    from jax.experimental.shard_map import shard_map
    from jax.sharding import Mesh, PartitionSpec

    from concourse.bass2jax import (
        _bass_exec_p,
        install_neuronx_cc_hook,
        partition_id_tensor,
    )

    install_neuronx_cc_hook()
    partition_name = nc.partition_id_tensor.name if nc.partition_id_tensor else None

    in_names, out_names, out_avals, zero_outs = [], [], [], []
    for alloc in nc.m.functions[0].allocations:
        if not isinstance(alloc, mybir.MemoryLocationSet):
            continue
        name = alloc.memorylocations[0].name
        if alloc.kind == "ExternalInput":
            if name != partition_name:
                in_names.append(name)
        elif alloc.kind == "ExternalOutput":
            shape = tuple(alloc.tensor_shape)
            dtype = mybir.dt.np(alloc.dtype)
            out_names.append(name)
            out_avals.append(jax.core.ShapedArray(shape, dtype))
            zero_outs.append(_np.zeros(shape, dtype))
    n_params = len(in_names)
    n_outs = len(out_avals)
    all_in_names = list(in_names) + list(out_names)
    if partition_name is not None:
        all_in_names.append(partition_name)

    def _call(operands):
        if partition_name is not None:
            operands = operands + [partition_id_tensor()]
        return _bass_exec_p.bind(
            *operands,
            out_avals=tuple(out_avals),
            in_names=tuple(all_in_names),
            out_names=tuple(out_names),
            lowering_input_output_aliases=(),
            sim_require_finite=True,
            sim_require_nnan=True,
            nc=nc,
        )

    if scan_n:
        assert n_outs == 1, "scan timing mode assumes a single output"

        def _body(*args):
            ins, carry = list(args[:n_params]), args[n_params]
            for _ in range(scan_n):
                (carry,) = _call(ins + [carry])
            return (carry,)

    else:

        def _body(*args):
            return tuple(_call(list(args)))

    devices = jax.devices()[:NCORES]
    mesh = Mesh(_np.asarray(devices), ("core",))
    in_specs = (PartitionSpec("core"),) * (n_params + n_outs)
    out_specs = (PartitionSpec("core"),) * n_outs
    jit_kwargs = dict(keep_unused=True)
    if donate:
        jit_kwargs["donate_argnums"] = tuple(range(n_params, n_params + n_outs))
    fn = jax.jit(
        shard_map(_body, mesh=mesh, in_specs=in_specs, out_specs=out_specs,
                  check_rep=False),
        **jit_kwargs,
    )

    def prep(in_maps):
        concat_in = [
            _np.concatenate([_np.asarray(m[name]) for m in in_maps], axis=0)
            for name in in_names
        ]
        concat_zeros = [
            _np.zeros((NCORES * z.shape[0], *z.shape[1:]), z.dtype)
            for z in zero_outs
        ]
        return concat_in, concat_zeros

    def unpack(out_arrs):
        return [
            {
                name: _np.asarray(out_arrs[i]).reshape(
                    NCORES, *out_avals[i].shape
                )[c]
                for i, name in enumerate(out_names)
            }
            for c in range(NCORES)
        ]

    return fn, prep, unpack


def kernel(
    x, pair, mask, norm_w, norm_b, qkv_w, qkv_b, qln_w, qln_b, kln_w, kln_b,
    proj_w, proj_b,
):
    global LAST_RESULTS
    x = np.asarray(x, dtype=np.float32)
    pair = np.asarray(pair, dtype=np.float32)
    mask = np.asarray(mask)
    f32 = np.float32
    f16 = np.float16

    # host-side weight folding
    w2 = (np.asarray(qkv_w, f32).T * np.asarray(norm_w, f32)[:, None]).astype(f32)
    b2 = np.asarray(qkv_b, f32) + np.asarray(norm_b, f32) @ np.asarray(qkv_w, f32).T
    assert np.all(b2 == 0.0), "nonzero effective qkv bias not supported"
    assert np.all(np.asarray(qln_b) == 0.0) and np.all(np.asarray(kln_b) == 0.0), (
        "nonzero q/k LN bias not supported"
    )
    assert np.all(np.asarray(proj_b) == 0.0), "nonzero proj bias not supported"
    w2cs = np.ascontiguousarray(
        w2.sum(axis=0, dtype=np.float64).astype(f32)[None, :]
    ).astype(f16)
    wp = np.ascontiguousarray(np.asarray(proj_w, f32).T).astype(f16)
    sc = (np.asarray(qln_w, f32) * np.asarray(kln_w, f32) * f32(SCALE)).astype(f32)
    w2h = w2.astype(f16)

    neg = np.float32(np.finfo(np.float32).min)
    in_maps = []
    for core in range(NCORES):
        b, ih = divmod(core, 2)
        i0 = ih * TQ
        # roll the sequence so this core's query rows are rows 0..TQ-1
        xb = np.concatenate([x[b, i0:], x[b, :i0]], axis=0)
        xT = np.ascontiguousarray(xb.T).astype(f16)
        # comb[h, j, i] = pair[b, i0+i, j, h] + (mask ? 0 : f32min), j rolled
        comb = np.ascontiguousarray(pair[b, i0 : i0 + TQ].transpose(2, 1, 0))
        mb = np.where(mask[b, i0 : i0 + TQ], f32(0.0), neg).T  # [j, i]
        comb += mb[None, :, :]
        comb = np.concatenate([comb[:, i0:, :], comb[:, :i0, :]], axis=1)
        # global logit shift (softmax-invariant) so exp() stays in fp16
        # range; masked entries clamp to a sentinel that still underflows
        # exp() to exactly 0 after +qk
        comb -= f32(LOGIT_SHIFT)
        comb = np.maximum(comb, -60000.0).astype(f16)
        comb = np.ascontiguousarray(comb)
        in_maps.append(
            {
                "xall": xb,
                "xT": xT,
                "comb": comb,
                "w2": w2h,
                "w2cs": w2cs,
                "wp": wp,
                "sc": sc,
                "eye": np.eye(P, dtype=f32),
                "eyeh": np.eye(P, dtype=f16),
                "ones": np.ones((P, H * H), f16),
            }
        )

    nc = _build()
    fn, prep, unpack = _make_runner(nc, donate=False)
    concat_in, concat_zeros = prep(in_maps)
    results = unpack(fn(*concat_in, *concat_zeros))
    LAST_RESULTS = {
        "nc": nc,
        "in_maps": in_maps,
        "fn": fn,
        "concat_in": concat_in,
        "concat_zeros": concat_zeros,
    }

    out = np.empty((B, T, C), dtype=np.float32)
    for core in range(NCORES):
        b, ih = divmod(core, 2)
        out[b, ih * TQ : (ih + 1) * TQ] = results[core]["o"]
    return out


# revision 27
# speedup vs baseline: 1.1891x; 1.0124x over previous
"""Trainium2 Bass kernel for nn_Attention_14508399525984 (sparse_attention).

Reference computation (B=4, T=1024, C=512, H=8, D=64):
    xn = LN(x, norm_w, norm_b)
    qkv = xn @ qkv_w.T + qkv_b ; q, k, v = split
    q = LN(q, qln_w, qln_b) ; k = LN(k, kln_w, kln_b)
    sim = (q @ k.T) * (D**-0.5) + pair.transpose(0,3,1,2) ; masked += f32min
    out = softmax(sim) @ v ; out @ proj_w.T + proj_b

Sharding: 8 cores = (batch b in 0..3) x (query half ih in 0..1).
Each core gets the full (rolled) batch-b sequence for k/v and its own 512
query rows; outputs are disjoint row blocks of the result.

Device kernel (per core). All matmul operands are fp16 (f32 PSUM accum):
fp16 reduces PE power draw (fp32r runs under a harsher HW power throttle)
and halves the weight/activation DMA. True HW exec time: ~147 us/core
(vs 194 us for the fp32r baseline), NTFF-profiled.
  - host folds norm_w into W2 = diag(norm_w) @ qkv_w.T (biases must be 0)
  - x LN is folded THROUGH the qkv matmul: qkv = rs*(x@W2) - rs*m*colsum(W2),
    realized as raw xT@W2 in PSUM plus a rank-1 (K=1) correction matmul;
    the rs row-scale is dropped for q/k (LN is scale-invariant) and applied
    at v eviction. The q-side matmuls/stats are skipped for the non-query
    half of the sequence (m >= 4). Inputs stream in chunked DMAs so the
    first qkv matmul starts after ~1.5 MB lands instead of ~9 MB.
  - q/k LN via bn_stats/bn_aggr on the PSUM; rsqrt = ACT Sqrt + one batched
    DVE reciprocal per chunk (covers x/k/q at once; a [P,1] DVE reciprocal
    costs ~1 us regardless of width, and the DMA-XBAR transpose path is
    serial ~1.2 us/tile, so both are avoided); normalize on ACT straight to
    fp16; PE fp16 transposes to [c, t]; the qln_w*kln_w*scale product is
    applied per-partition on DVE at q eviction.
  - attention computed transposed: simT[j,i] via K=64 matmuls whose 0/64
    base partitions walrus auto-row-tiles into disjoint PE row groups (both
    heads' sims run concurrently); pair bias + mask tile (comb, fp16,
    host-precomputed with a -4 logit shift so exp fits fp16) added on DVE
    straight to fp16, exp in place on ACT (masked entries are ~-6e4 ->
    exp == 0, no max-subtraction needed); the PV matmul carries a
    ones-column on v giving rows 0..63 = (E@v).T and row 64 = sum_j E.
  - normalize by the sum row: evict to SBUF (ACT), DMA-scatter the row
    across 128 partitions for a cheap DVE reciprocal ([1,512] on a single
    partition costs ~3 us), DMA-gather back, gpsimd partition-broadcast,
    DVE multiply; assemble attn.T [c, i] fp16, project with proj_w.T;
    per-chunk output DMA.
"""

import numpy as np

import concourse.bacc as bacc
import concourse.tile as tile
from concourse import mybir
from concourse.bass_utils import run_bass_kernel_spmd

B, T, C, H, D = 4, 1024, 512, 8, 64
EPS = 1e-5
SCALE = float(D) ** -0.5  # TEMP = 1.0
LOGIT_SHIFT = 4.0  # host subtracts from comb; softmax is shift-invariant
TQ = T // 2  # query rows per core
NCORES = 8
P = 128
F32 = mybir.dt.float32
F16 = mybir.dt.float16

LAST_RESULTS = None  # test harness peeks at this for exec_time_ns


def _build(phases=("ab", "attn", "proj")):
    import os
    phases = tuple(os.environ.get("KPHASES", ",".join(phases)).split(","))
    nc = bacc.Bacc(
        "TRN2",
        target_bir_lowering=False,
        debug=False,
        enable_asserts=False,
        num_devices=NCORES,
    )
    xall_d = nc.declare_dram_parameter("xall", [T, C], F32, isOutput=False)
    xT_d = nc.declare_dram_parameter("xT", [C, T], F16, isOutput=False)
    comb_d = nc.declare_dram_parameter("comb", [H, T, TQ], F16, isOutput=False)
    w2_d = nc.declare_dram_parameter("w2", [C, 3 * C], F16, isOutput=False)
    w2cs_d = nc.declare_dram_parameter("w2cs", [1, 3 * C], F16, isOutput=False)
    wp_d = nc.declare_dram_parameter("wp", [C, C], F16, isOutput=False)
    sc_d = nc.declare_dram_parameter("sc", [C], F32, isOutput=False)
    eye_d = nc.declare_dram_parameter("eye", [P, P], F32, isOutput=False)
    eyeh_d = nc.declare_dram_parameter("eyeh", [P, P], F16, isOutput=False)
    ones_d = nc.declare_dram_parameter("ones", [P, H * H], F16, isOutput=False)
    o_d = nc.declare_dram_parameter("o", [TQ, C], F16, isOutput=True)

    from contextlib import ExitStack

    with tile.TileContext(nc) as tc, ExitStack() as ctx:
        consts = ctx.enter_context(tc.tile_pool(name="consts", bufs=1))
        work = ctx.enter_context(tc.tile_pool(name="work", bufs=4))
        evp = ctx.enter_context(tc.tile_pool(name="evp", bufs=3))

        # tiny consts first, then the big input tiles chunked so the first
        # A/B iteration can start after ~1.5 MB instead of ~9 MB
        ident = consts.tile([P, P], F32)
        nc.sync.dma_start(out=ident, in_=eye_d[:, :])
        identh = consts.tile([P, P], F16)
        nc.sync.dma_start(out=identh, in_=eyeh_d[:, :])
        eps_t = consts.tile([P, 1], F32)
        nc.vector.memset(eps_t, EPS)
        sc_sb = consts.tile([P, 4], F32)
        nc.sync.dma_start(out=sc_sb, in_=sc_d.rearrange("(c p) -> p c", p=P))
        w2cs_sb = consts.tile([1, 3 * C], F16)
        nc.sync.dma_start(out=w2cs_sb, in_=w2cs_d[:, :])

        x_sb = consts.tile([P, 8, C], F32)
        xT_sb = consts.tile([P, 4, T], F16)
        w2_sb = consts.tile([P, 4, 3 * C], F16)
        xall_r = xall_d.rearrange("(m p) c -> p m c", p=P)
        xT_r = xT_d.rearrange("(kc p) t -> p kc t", p=P)
        w2_r = w2_d.rearrange("(kc p) n -> p kc n", p=P)
        nc.sync.dma_start(out=x_sb[:, 0, :], in_=xall_r[:, 0, :])
        nc.sync.dma_start(out=xT_sb[:, :, 0:P], in_=xT_r[:, :, 0:P])
        for cc in range(4):
            nc.sync.dma_start(out=w2_sb[:, cc, :], in_=w2_r[:, cc, :])
        for m in range(1, 8):
            ms = slice(m * P, (m + 1) * P)
            nc.sync.dma_start(out=x_sb[:, m, :], in_=xall_r[:, m, :])
            nc.sync.dma_start(out=xT_sb[:, :, ms], in_=xT_r[:, :, ms])

        wp_sb = consts.tile([P, 4, C], F16)
        nc.sync.dma_start(out=wp_sb, in_=wp_d.rearrange("(kc p) n -> p kc n", p=P))

        qT_sb = consts.tile([P, 4, TQ], F16)  # [c, i] query half, sc-scaled
        kT_sb = consts.tile([P, 4, T], F16)  # [c, j]
        v_sb = consts.tile([P, 8, H, D + 1], F16)  # [j_part, jc, h, d | ones]
        nc.sync.dma_start(
            out=v_sb[:, :, :, D],
            in_=ones_d.rearrange("p (a b) -> p a b", a=8),
        )
        attnT_sb = consts.tile([P, 4, TQ], F16)  # [c, i] normalized attn out
        o_sb = consts.tile([P, 4, C], F16)

        # ---------------- phase A/B: LN + qkv + transposes ----------------
        if "ab" in phases:
         with tc.tile_pool(name="pT", bufs=2, space="PSUM") as pT, tc.tile_pool(
            name="pQ", bufs=2, space="PSUM"
        ) as pQ:
            def stats_sd(src, sdall, col, tag):
                """bn stats + sqrt(var+eps) into a shared column of sdall so
                one batched DVE reciprocal serves all stats of the m-chunk
                (a [P,1] DVE reciprocal costs ~1us regardless of width)."""
                st = work.tile([P, 6], F32, name=f"st{tag}")
                nc.vector.bn_stats(st, src)
                mv = work.tile([P, 2], F32, name=f"mv{tag}")
                nc.vector.bn_aggr(mv, st)
                nc.scalar.activation(
                    sdall[:, col : col + 1], mv[:, 1:2],
                    mybir.ActivationFunctionType.Sqrt, bias=eps_t,
                )
                return mv

            def emit_transposes(m, kn, qn):
                ms = slice(m * P, (m + 1) * P)
                if m == 7:
                    # last chunk: nothing left to overlap the PE transposes
                    # with, and the in-order PE stream would stall the first
                    # attention sims behind them. The DMA XBAR (slow but off
                    # the PE stream) hides under the early attention
                    # iterations; only the jc=7 sims need this kT chunk.
                    for cc in range(4):
                        nc.sync.dma_start(
                            out=kT_sb[:, cc, ms],
                            in_=kn[:, cc * P : (cc + 1) * P],
                            transpose=True,
                        )
                    return
                ptk = pT.tile([P, 4, P], F16, name="ptk", tag="tp")
                for cc in range(4):
                    nc.tensor.transpose(
                        ptk[:, cc, :], kn[:, cc * P : (cc + 1) * P], identh
                    )
                nc.scalar.copy(out=kT_sb[:, :, ms], in_=ptk)
                if qn is not None:
                    ptq = pT.tile([P, 4, P], F16, name="ptq", tag="tp")
                    for cc in range(4):
                        nc.tensor.transpose(
                            ptq[:, cc, :], qn[:, cc * P : (cc + 1) * P], identh
                        )
                    # eviction + qln_w*kln_w*scale fold in one DVE pass
                    for cc in range(4):
                        nc.vector.tensor_scalar_mul(
                            out=qT_sb[:, cc, ms],
                            in0=ptq[:, cc, :],
                            scalar1=sc_sb[:, cc : cc + 1],
                        )

            pend = None
            for m in range(8):
                ms = slice(m * P, (m + 1) * P)
                has_q = m < 4
                nstat = 3 if has_q else 2
                sdall = work.tile([P, 3], F32, name="sdall")
                rsall = work.tile([P, 3], F32, name="rsall")
                # x row stats for this t-chunk (gates only the rank-1 mms)
                mv = stats_sd(x_sb[:, m, :], sdall, 0, "x")
                negrm = work.tile([P, 1], F32, name="negrm")
                nc.vector.tensor_scalar_mul(out=negrm, in0=mv[:, 0:1], scalar1=-1.0)

                # qkv: psum[t, n] = xT.T @ W2 + negrm x colsum(W2)
                # (missing rs row-scale; q/k LN is scale-invariant, v gets
                # rs at eviction)
                ps_k = pQ.tile([P, C], F32, name="ps_k")
                ps_v = pQ.tile([P, C], F32, name="ps_v")
                ps_q = pQ.tile([P, C], F32, name="ps_q") if has_q else None
                for cc in range(4):
                    lw = xT_sb[:, cc, ms]
                    if has_q:
                        nc.tensor.matmul(
                            ps_q, lw, w2_sb[:, cc, 0:C], start=(cc == 0), stop=False
                        )
                    nc.tensor.matmul(
                        ps_k, lw, w2_sb[:, cc, C : 2 * C], start=(cc == 0), stop=False
                    )
                    nc.tensor.matmul(
                        ps_v, lw, w2_sb[:, cc, 2 * C : 3 * C],
                        start=(cc == 0), stop=False,
                    )
                # negrm as a row [1, 128] for the rank-1 correction
                nr_ps = pT.tile([1, P], F32, name="nr_ps", tag="tp")
                nc.tensor.transpose(nr_ps, negrm, ident)
                nr = work.tile([1, P], F16, name="nr")
                nc.scalar.copy(out=nr, in_=nr_ps)
                if has_q:
                    nc.tensor.matmul(
                        ps_q, nr, w2cs_sb[:, 0:C], start=False, stop=True
                    )
                nc.tensor.matmul(
                    ps_k, nr, w2cs_sb[:, C : 2 * C], start=False, stop=True
                )
                nc.tensor.matmul(
                    ps_v, nr, w2cs_sb[:, 2 * C : 3 * C], start=False, stop=True
                )

                # ---- k path ----
                mvk = stats_sd(ps_k, sdall, 1, "k")
                if has_q:
                    mvq = stats_sd(ps_q, sdall, 2, "q")
                nc.vector.reciprocal(rsall[:, 0:nstat], sdall[:, 0:nstat])
                rs = rsall[:, 0:1]
                rsk = rsall[:, 1:2]
                nmk = work.tile([P, 1], F32, name="nmk")
                nc.vector.tensor_scalar(
                    out=nmk, in0=mvk[:, 0:1], scalar1=rsk, scalar2=-1.0,
                    op0=mybir.AluOpType.mult, op1=mybir.AluOpType.mult,
                )
                kn = evp.tile([P, C], F16, name="kn")
                nc.scalar.activation(
                    kn, ps_k, mybir.ActivationFunctionType.Identity,
                    bias=nmk, scale=rsk,
                )

                qn = None
                if has_q:
                    rsq = rsall[:, 2:3]
                    nmq = work.tile([P, 1], F32, name="nmq")
                    nc.vector.tensor_scalar(
                        out=nmq, in0=mvq[:, 0:1], scalar1=rsq, scalar2=-1.0,
                        op0=mybir.AluOpType.mult, op1=mybir.AluOpType.mult,
                    )
                    qn = evp.tile([P, C], F16, name="qn")
                    nc.scalar.activation(
                        qn, ps_q, mybir.ActivationFunctionType.Identity,
                        bias=nmq, scale=rsq,
                    )

                # ---- v path: v = rs * psum, into [j, jc, h, d] with ones col
                nc.scalar.activation(
                    v_sb[:, m, :, 0:D],
                    ps_v.rearrange("p (h d) -> p h d", h=H),
                    mybir.ActivationFunctionType.Copy,
                    scale=rs,
                )

                # software-pipeline: the transposes wait ~3us on the stats ->
                # normalize chain, and the in-order PE stream would stall the
                # next m's qkv matmuls behind them. Emit the PREVIOUS chunk's
                # transposes here instead (its kn/qn landed during this
                # chunk's matmuls), keeping the PE fed.
                if pend is not None:
                    emit_transposes(*pend)
                pend = (m, kn, qn)

            emit_transposes(*pend)

        # ---------------- attention ----------------
        if "attn" in phases:
         with tc.tile_pool(name="pS", bufs=2, space="PSUM") as pS, tc.tile_pool(
            name="pV", bufs=2, space="PSUM"
        ) as pV, tc.tile_pool(name="combp", bufs=10) as combp, tc.tile_pool(
            name="ep", bufs=3
        ) as ep, tc.tile_pool(name="fin", bufs=2) as fin:
            for hp in range(4):
                h0, h1 = 2 * hp, 2 * hp + 1
                pv0 = pV.tile([D + 1, TQ], F32, name="pv0")
                pv1 = pV.tile([D + 1, TQ], F32, name="pv1")
                for jc in range(8):
                    js = slice(jc * P, (jc + 1) * P)
                    # both heads of the pair batched into one wide tile:
                    # one DMA, one DVE add, one ACT exp per (hp, jc)
                    cmb = combp.tile([P, 2, TQ], F16, name="cmb")
                    nc.sync.dma_start(
                        out=cmb,
                        in_=comb_d[h0 : h0 + 2, js, :].transpose([1, 0, 2]),
                    )
                    sim = pS.tile([P, 2, TQ], F32, name="sim")
                    # K=64 each with base partitions 0/64: walrus row-tiles
                    # the pair into disjoint PE row-groups automatically, so
                    # both heads' sims run concurrently
                    for idx in range(2):
                        lo, hi = (0, D) if idx == 0 else (D, 2 * D)
                        nc.tensor.matmul(
                            sim[:, idx, :],
                            kT_sb[lo:hi, hp, js],
                            qT_sb[lo:hi, hp, :],
                            start=True,
                            stop=True,
                        )
                    # add straight to fp16, exp in place (fp16 logits are
                    # fine: |logit| <= ~10, abs err ~5e-3)
                    et = ep.tile([P, 2, TQ], F16, name="et")
                    nc.vector.tensor_add(out=et, in0=sim, in1=cmb)
                    nc.scalar.activation(
                        et, et, mybir.ActivationFunctionType.Exp
                    )
                    for idx, (h, pvt) in enumerate(((h0, pv0), (h1, pv1))):
                        nc.tensor.matmul(
                            pvt,
                            v_sb[:, jc, h, :],
                            et[:, idx, :],
                            start=(jc == 0),
                            stop=(jc == 7),
                        )
                # finalize both heads: divide by the sum row
                for idx, pvt in enumerate((pv0, pv1)):
                    # evict the sum row to SBUF (ACT, stays on partition
                    # 64), DMA-scatter it across 128 partitions for the
                    # reciprocal (a [1,512] single-partition DVE op costs
                    # ~3us; [128,4] costs ~0.1us), DMA-gather back to a
                    # row, broadcast on gpsimd
                    srow = fin.tile([D + 1, TQ], F32, name=f"srow{idx}")
                    nc.scalar.copy(out=srow[D : D + 1, :], in_=pvt[D : D + 1, :])
                    s4 = fin.tile([P, 4], F32, name=f"s4{idx}")
                    nc.sync.dma_start(out=s4, in_=srow[D : D + 1, :])
                    r4 = fin.tile([P, 4], F32, name=f"r4{idx}")
                    nc.vector.reciprocal(r4, s4)
                    r0 = fin.tile([1, TQ], F32, name=f"r0{idx}")
                    nc.sync.dma_start(out=r0, in_=r4)
                    rb = fin.tile([D, TQ], F32, name=f"rb{idx}")
                    nc.gpsimd.partition_broadcast(rb, r0)
                    if idx == 0:
                        nc.vector.tensor_mul(
                            out=attnT_sb[0:D, hp, :], in0=pvt[0:D, :], in1=rb
                        )
                    else:
                        tmo = fin.tile([D, TQ], F16, name="tmo")
                        nc.vector.tensor_mul(out=tmo, in0=pvt[0:D, :], in1=rb)
                        nc.sync.dma_start(out=attnT_sb[D:P, hp, :], in_=tmo)

        # ---------------- projection ----------------
        if "proj" in phases:
         with tc.tile_pool(name="pO", bufs=1, space="PSUM") as pO:
            o_r = o_d.rearrange("(ic p) n -> p ic n", p=P)
            # cc (= head pair) as the OUTER loop over 4 live PSUM tiles:
            # 12 of the 16 matmuls depend only on head pairs 0..2 and run
            # while the last head pair's softmax finalize is still in
            # flight (ic-outer order would stall the in-order PE stream on
            # the first ic's cc=3 matmul)
            po_t = [pO.tile([P, C], F32, name=f"po{ic}") for ic in range(4)]
            for cc in range(4):
                for ic in range(4):
                    nc.tensor.matmul(
                        po_t[ic],
                        attnT_sb[:, cc, ic * P : (ic + 1) * P],
                        wp_sb[:, cc, :],
                        start=(cc == 0),
                        stop=(cc == 3),
                    )
                    if cc == 3:
                        nc.scalar.copy(out=o_sb[:, ic, :], in_=po_t[ic])
                        nc.sync.dma_start(out=o_r[:, ic, :], in_=o_sb[:, ic, :])

    nc.compile()
    return nc


def _make_runner(nc, donate=True, scan_n=0):
    """Mirror of bass2jax.run_bass_via_pjrt that returns a reusable jitted
    callable (so the harness can time repeated executions on-device).

    scan_n > 0 chains scan_n sequential executions of the NEFF inside one
    dispatch (the output feeds the next iteration's output-donation operand),
    letting wall-clock deltas isolate the per-execution device time from the
    axon dispatch overhead."""
    import jax
    import numpy as _np
    from jax.experimental.shard_map import shard_map
    from jax.sharding import Mesh, PartitionSpec

    from concourse.bass2jax import (
        _bass_exec_p,
        install_neuronx_cc_hook,
        partition_id_tensor,
    )

    install_neuronx_cc_hook()
    partition_name = nc.partition_id_tensor.name if nc.partition_id_tensor else None

    in_names, out_names, out_avals, zero_outs = [], [], [], []
    for alloc in nc.m.functions[0].allocations:
        if not isinstance(alloc, mybir.MemoryLocationSet):
            continue
        name = alloc.memorylocations[0].name
        if alloc.kind == "ExternalInput":
            if name != partition_name:
                in_names.append(name)
        elif alloc.kind == "ExternalOutput":
            shape = tuple(alloc.tensor_shape)
            dtype = mybir.dt.np(alloc.dtype)
            out_names.append(name)
            out_avals.append(jax.core.ShapedArray(shape, dtype))
            zero_outs.append(_np.zeros(shape, dtype))
    n_params = len(in_names)
    n_outs = len(out_avals)
    all_in_names = list(in_names) + list(out_names)
    if partition_name is not None:
        all_in_names.append(partition_name)

    def _call(operands):
        if partition_name is not None:
            operands = operands + [partition_id_tensor()]
        return _bass_exec_p.bind(
            *operands,
            out_avals=tuple(out_avals),
            in_names=tuple(all_in_names),
            out_names=tuple(out_names),
            lowering_input_output_aliases=(),
            sim_require_finite=True,
            sim_require_nnan=True,
            nc=nc,
        )

    if scan_n:
        assert n_outs == 1, "scan timing mode assumes a single output"

        def _body(*args):
            ins, carry = list(args[:n_params]), args[n_params]
            for _ in range(scan_n):
                (carry,) = _call(ins + [carry])
            return (carry,)

    else:

        def _body(*args):
            return tuple(_call(list(args)))

    devices = jax.devices()[:NCORES]
    mesh = Mesh(_np.asarray(devices), ("core",))
    in_specs = (PartitionSpec("core"),) * (n_params + n_outs)
    out_specs = (PartitionSpec("core"),) * n_outs
    jit_kwargs = dict(keep_unused=True)
    if donate:
        jit_kwargs["donate_argnums"] = tuple(range(n_params, n_params + n_outs))
    fn = jax.jit(
        shard_map(_body, mesh=mesh, in_specs=in_specs, out_specs=out_specs,
                  check_rep=False),
        **jit_kwargs,
    )

    def prep(in_maps):
        concat_in = [
            _np.concatenate([_np.asarray(m[name]) for m in in_maps], axis=0)
            for name in in_names
        ]
        concat_zeros = [
            _np.zeros((NCORES * z.shape[0], *z.shape[1:]), z.dtype)
            for z in zero_outs
        ]
        return concat_in, concat_zeros

    def unpack(out_arrs):
        return [
            {
                name: _np.asarray(out_arrs[i]).reshape(
                    NCORES, *out_avals[i].shape
                )[c]
                for i, name in enumerate(out_names)
            }
            for c in range(NCORES)
        ]

    return fn, prep, unpack


def kernel(
    x, pair, mask, norm_w, norm_b, qkv_w, qkv_b, qln_w, qln_b, kln_w, kln_b,
    proj_w, proj_b,
):
    global LAST_RESULTS
    x = np.asarray(x, dtype=np.float32)
    pair = np.asarray(pair, dtype=np.float32)
    mask = np.asarray(mask)
    f32 = np.float32
    f16 = np.float16

    # host-side weight folding
    w2 = (np.asarray(qkv_w, f32).T * np.asarray(norm_w, f32)[:, None]).astype(f32)
    b2 = np.asarray(qkv_b, f32) + np.asarray(norm_b, f32) @ np.asarray(qkv_w, f32).T
    assert np.all(b2 == 0.0), "nonzero effective qkv bias not supported"
    assert np.all(np.asarray(qln_b) == 0.0) and np.all(np.asarray(kln_b) == 0.0), (
        "nonzero q/k LN bias not supported"
    )
    assert np.all(np.asarray(proj_b) == 0.0), "nonzero proj bias not supported"
    w2cs = np.ascontiguousarray(
        w2.sum(axis=0, dtype=np.float64).astype(f32)[None, :]
    ).astype(f16)
    wp = np.ascontiguousarray(np.asarray(proj_w, f32).T).astype(f16)
    sc = (np.asarray(qln_w, f32) * np.asarray(kln_w, f32) * f32(SCALE)).astype(f32)
    w2h = w2.astype(f16)

    neg = np.float32(np.finfo(np.float32).min)
    in_maps = []
    for core in range(NCORES):
        b, ih = divmod(core, 2)
        i0 = ih * TQ
        # roll the sequence so this core's query rows are rows 0..TQ-1
        xb = np.concatenate([x[b, i0:], x[b, :i0]], axis=0)
        xT = np.ascontiguousarray(xb.T).astype(f16)
        # comb[h, j, i] = pair[b, i0+i, j, h] + (mask ? 0 : f32min), j rolled
        comb = np.ascontiguousarray(pair[b, i0 : i0 + TQ].transpose(2, 1, 0))
        mb = np.where(mask[b, i0 : i0 + TQ], f32(0.0), neg).T  # [j, i]
        comb += mb[None, :, :]
        comb = np.concatenate([comb[:, i0:, :], comb[:, :i0, :]], axis=1)
        # global logit shift (softmax-invariant) so exp() stays in fp16
        # range; masked entries clamp to a sentinel that still underflows
        # exp() to exactly 0 after +qk
        comb -= f32(LOGIT_SHIFT)
        comb = np.maximum(comb, -60000.0).astype(f16)
        comb = np.ascontiguousarray(comb)
        in_maps.append(
            {
                "xall": xb,
                "xT": xT,
                "comb": comb,
                "w2": w2h,
                "w2cs": w2cs,
                "wp": wp,
                "sc": sc,
                "eye": np.eye(P, dtype=f32),
                "eyeh": np.eye(P, dtype=f16),
                "ones": np.ones((P, H * H), f16),
            }
        )

    nc = _build()
    fn, prep, unpack = _make_runner(nc, donate=False)
    concat_in, concat_zeros = prep(in_maps)
    results = unpack(fn(*concat_in, *concat_zeros))
    LAST_RESULTS = {
        "nc": nc,
        "in_maps": in_maps,
        "fn": fn,
        "concat_in": concat_in,
        "concat_zeros": concat_zeros,
    }

    out = np.empty((B, T, C), dtype=np.float32)
    for core in range(NCORES):
        b, ih = divmod(core, 2)
        out[b, ih * TQ : (ih + 1) * TQ] = results[core]["o"].astype(np.float32)
    return out
